# revision 1
# baseline (speedup 1.0000x reference)
"""HGTConv Trainium2 kernel (8 NeuronCores, dst-sharded edge parallel).

Math: in the reference, softmax over the H=8 head axis followed by
attn.mean(axis=-1) is identically 1/8, so the whole attention branch
(K/Q projections, Wa) drops out:

    out_dst = relu( (segsum_dst(x_src[src]) @ Wbig + cnt*bbig + 8*max(cnt,1)*bout)
                    / (8*max(cnt,1)) + x_dst )
    Wbig = Wv @ Wm @ Wout,  bbig = (bv @ Wm + bm) @ Wout

Sharding: each core owns a contiguous dst-node range (1/8 of users +
1/8 of games) and receives exactly the edges pointing into it, so no
collectives are needed. Node features are replicated (bf16 for the
gather path, f32 slices for the residual).

Device per dst tile (128 nodes): one-hot matmul scatter-add. Edges are
host-packed into chunks of 128; a [128e,128d] selection matrix M
(built on DVE from local-dst values vs an iota row) turns the
segment-sum into PE matmuls accumulating S^T in PSUM, then the fused
(Wbig|bbig|bout) matmul, row-scaling by 1/(8*max(cnt,1)), residual add
and relu.
"""

import math
from contextlib import ExitStack

import numpy as np
import ml_dtypes

import concourse.bass as bass
import concourse.tile as tile
import concourse.mybir as mybir
from concourse import bacc
from concourse.bass_utils import run_bass_kernel_spmd

P = 128
D = 256
BF16 = ml_dtypes.bfloat16
DUMMY_IDX = 0  # dummy slots gather row 0; M-matrix zeroes their contribution

# full-size problem config
CFG_FULL = dict(n_user=100000, n_game=50000, ncores=8, cu=3, cg=5)


def _cfg_derived(cfg):
    ncores = cfg["ncores"]
    uslice = cfg["n_user"] // ncores
    gslice = cfg["n_game"] // ncores
    ut = math.ceil(uslice / P)
    gt = math.ceil(gslice / P)
    return uslice, gslice, ut, gt


# ----------------------------------------------------------------- host prep

def _pack_side(src, dst, lo, hi, n_tiles, C):
    """Edges with dst in [lo, hi) packed into per-dst-tile chunks of 128.

    Returns idx [P, n_tiles*C] int32 (src row ids, dummy slots -> 0),
    ld [P, n_tiles*C] bf16 (dst offset within tile 0..127, dummy -> -1),
    ch [2, n_tiles*P] bf16 (row0 = per-node edge count, row1 = 8*max(cnt,1)),
    r8 [P, n_tiles] f32 (1 / (8*max(cnt,1)), partition-major).
    """
    sel = (dst >= lo) & (dst < hi)
    s = src[sel].astype(np.int64)
    d = (dst[sel] - lo).astype(np.int64)
    order = np.argsort(d, kind="stable")
    s = s[order]
    d = d[order]
    tile_of = d >> 7
    bounds = np.searchsorted(tile_of, np.arange(n_tiles + 1))
    idx = np.full((P, n_tiles * C), DUMMY_IDX, np.int32)
    ld = np.full((P, n_tiles * C), -1.0, dtype=np.float32)
    for t in range(n_tiles):
        a, b = int(bounds[t]), int(bounds[t + 1])
        cnt_t = b - a
        if cnt_t == 0:
            continue
        assert cnt_t <= C * P, f"dst tile overflow: {cnt_t} edges > {C * P} slots"
        j = np.arange(cnt_t)
        idx[j % P, t * C + j // P] = s[a:b]
        ld[j % P, t * C + j // P] = (d[a:b] - t * P).astype(np.float32)

    cnt = np.bincount(d, minlength=n_tiles * P).astype(np.float32)
    m8 = 8.0 * np.maximum(cnt, 1.0)
    ch = np.stack([cnt, m8], axis=0).astype(BF16)             # [2, T*P]
    r8 = np.ascontiguousarray((1.0 / m8).reshape(n_tiles, P).T.astype(np.float32))
    return idx, ld.astype(BF16), ch, r8


def _fold_weights(Wv, bv, Wm, bm, Wout, bout):
    Wbig = (np.float32(Wv) @ np.float32(Wm)) @ np.float32(Wout)
    bbig = (np.float32(bv) @ np.float32(Wm) + np.float32(bm)) @ np.float32(Wout)
    w = np.concatenate([Wbig, bbig[None, :], np.float32(bout)[None, :]], axis=0)
    return np.ascontiguousarray(w).astype(BF16)  # [D+2, D]


# ------------------------------------------------------------- device build

def _build(cfg):
    uslice, gslice, ut, gt = _cfg_derived(cfg)
    cu, cg = cfg["cu"], cfg["cg"]
    f32 = mybir.dt.float32
    bf = mybir.dt.bfloat16
    i32 = mybir.dt.int32

    nc = bacc.Bacc(
        "TRN2",
        target_bir_lowering=False,
        debug=False,
        num_devices=cfg["ncores"],
    )

    xu_bf = nc.dram_tensor("xu_bf", [cfg["n_user"], D], bf, kind="ExternalInput")
    xg_bf = nc.dram_tensor("xg_bf", [cfg["n_game"], D], bf, kind="ExternalInput")

    sides = []
    for name, tiles, C, xsrc, nsrc in (
        ("u", ut, cu, xg_bf, cfg["n_game"]),
        ("g", gt, cg, xu_bf, cfg["n_user"]),
    ):
        side = dict(name=name, tiles=tiles, C=C, xsrc=xsrc, nsrc=nsrc)
        side["xres"] = nc.dram_tensor(f"xres_{name}", [P, tiles * D], f32, kind="ExternalInput")
        side["idx"] = nc.dram_tensor(f"idx_{name}", [P, tiles * C], i32, kind="ExternalInput")
        side["ld"] = nc.dram_tensor(f"ld_{name}", [P, tiles * C], bf, kind="ExternalInput")
        side["ch"] = nc.dram_tensor(f"ch_{name}", [2, tiles * P], bf, kind="ExternalInput")
        side["r8"] = nc.dram_tensor(f"r8_{name}", [P, tiles], f32, kind="ExternalInput")
        side["w"] = nc.dram_tensor(f"w_{name}", [D + 2, D], bf, kind="ExternalInput")
        side["out"] = nc.dram_tensor(f"out_{name}", [P, tiles * D], f32, kind="ExternalOutput")
        sides.append(side)

    with tile.TileContext(nc) as tc, ExitStack() as ctx:
        const = ctx.enter_context(tc.tile_pool(name="const", bufs=1))
        gx = ctx.enter_context(tc.tile_pool(name="gx", bufs=3))
        mp = ctx.enter_context(tc.tile_pool(name="mp", bufs=4))
        stp = ctx.enter_context(tc.tile_pool(name="stp", bufs=4))
        xrp = ctx.enter_context(tc.tile_pool(name="xrp", bufs=3))
        outp = ctx.enter_context(tc.tile_pool(name="outp", bufs=3))
        st_ps = ctx.enter_context(tc.tile_pool(name="st_ps", bufs=4, space="PSUM"))
        op_ps = ctx.enter_context(tc.tile_pool(name="op_ps", bufs=3, space="PSUM"))

        for i in range(3):
            Xp = gx.tile([P, D], bf, tag="gx", name=f"gx_prime_{i}")
            nc.vector.memset(Xp[:], 0.0)

        # constants
        iota_bf = const.tile([P, P], bf)
        nc.gpsimd.iota(
            iota_bf[:], pattern=[[1, P]], base=0, channel_multiplier=0,
            allow_small_or_imprecise_dtypes=True,
        )

        for side in sides:
            T, C = side["tiles"], side["C"]
            n = side["name"]
            side["idx_res"] = const.tile([P, T * C], i32, tag=f"idx_{n}", name=f"idx_res_{n}")
            nc.sync.dma_start(side["idx_res"][:], side["idx"][:])
            side["ld_res"] = const.tile([P, T * C], bf, tag=f"ld_{n}", name=f"ld_res_{n}")
            nc.sync.dma_start(side["ld_res"][:], side["ld"][:])
            side["ch_res"] = const.tile([2, T * P], bf, tag=f"ch_{n}", name=f"ch_res_{n}")
            nc.sync.dma_start(side["ch_res"][:], side["ch"][:])
            side["r8_res"] = const.tile([P, T], f32, tag=f"r8_{n}", name=f"r8_res_{n}")
            nc.sync.dma_start(side["r8_res"][:], side["r8"][:])
            side["w0"] = const.tile([P, D], bf, tag=f"w0_{n}", name=f"w0_{n}")
            nc.sync.dma_start(side["w0"][:], side["w"][0:P, :])
            side["w1"] = const.tile([P, D], bf, tag=f"w1_{n}", name=f"w1_{n}")
            nc.sync.dma_start(side["w1"][:], side["w"][P : 2 * P, :])
            side["w2"] = const.tile([2, D], bf, tag=f"w2_{n}", name=f"w2_{n}")
            nc.sync.dma_start(side["w2"][:], side["w"][2 * P : 2 * P + 2, :])

        for side in sides:
            T, C = side["tiles"], side["C"]
            idx_res, ld_res = side["idx_res"], side["ld_res"]
            G = max(g for g in range(1, 9) if T % g == 0)
            xr_grp = og_grp = None
            for t in range(T):
                gi = t % G
                if gi == 0:
                    xr_grp = xrp.tile([P, G * D], f32, tag="xr", name="xr_grp")
                    nc.sync.dma_start(xr_grp[:], side["xres"][:, t * D : (t + G) * D])
                    og_grp = outp.tile([P, G * D], f32, tag="og", name="og_grp")
                base = t * C
                st0_ps = st_ps.tile([P, P], f32, tag="st")
                st1_ps = st_ps.tile([P, P], f32, tag="st")
                for c in range(C):
                    # gather the 128 source rows for this edge chunk
                    # (HW indirect DMA: one index per partition-row)
                    X = gx.tile([P, D], bass.mybir.dt.bfloat16, tag="gx")
                    nc.gpsimd.indirect_dma_start(
                        out=X[:],
                        out_offset=None,
                        in_=side["xsrc"][:],
                        in_offset=bass.IndirectOffsetOnAxis(
                            ap=idx_res[:, base + c : base + c + 1], axis=0
                        ),
                    )
                    Mt = mp.tile([P, P], bf, tag="m")
                    nc.vector.tensor_tensor(
                        out=Mt[:],
                        in0=ld_res[:, base + c : base + c + 1].to_broadcast([P, P]),
                        in1=iota_bf[:],
                        op=mybir.AluOpType.is_equal,
                    )
                    s, e = (c == 0), (c == C - 1)
                    nc.tensor.matmul(st0_ps[:], lhsT=X[:, 0:P], rhs=Mt[:], start=s, stop=e)
                    nc.tensor.matmul(st1_ps[:], lhsT=X[:, P:D], rhs=Mt[:], start=s, stop=e)
                # finish
                st0 = stp.tile([P, P], bf, tag="st")
                nc.scalar.copy(st0[:], st0_ps[:])
                st1 = stp.tile([P, P], bf, tag="st")
                nc.vector.tensor_copy(st1[:], st1_ps[:])

                opre = op_ps.tile([P, D], f32, tag="opre")
                nc.tensor.matmul(opre[:], lhsT=st0[:], rhs=side["w0"][:], start=True, stop=False)
                nc.tensor.matmul(opre[:], lhsT=st1[:], rhs=side["w1"][:], start=False, stop=False)
                nc.tensor.matmul(
                    opre[:], lhsT=side["ch_res"][:, t * P : (t + 1) * P],
                    rhs=side["w2"][:], start=False, stop=True,
                )

                tt = outp.tile([P, D], f32, tag="tt")
                nc.scalar.activation(
                    tt[:], opre[:], mybir.ActivationFunctionType.Copy,
                    scale=side["r8_res"][:, t : t + 1],
                )
                oo = outp.tile([P, D], f32, tag="oo")
                nc.vector.tensor_tensor(
                    out=oo[:], in0=tt[:], in1=xr_grp[:, gi * D : (gi + 1) * D],
                    op=mybir.AluOpType.add,
                )
                nc.scalar.activation(
                    og_grp[:, gi * D : (gi + 1) * D], oo[:],
                    mybir.ActivationFunctionType.Relu,
                )
                if gi == G - 1:
                    nc.sync.dma_start(
                        side["out"][:, (t - G + 1) * D : (t + 1) * D], og_grp[:]
                    )

    nc.compile()
    return nc


_NC_CACHE = {}


def _get_nc(cfg):
    key = tuple(sorted(cfg.items()))
    if key not in _NC_CACHE:
        _NC_CACHE[key] = _build(cfg)
    return _NC_CACHE[key]


# ------------------------------------------------------------------- driver

def _make_in_maps(cfg, x_user, x_game, w_user, w_game,
                  ei_played_src, ei_played_dst, ei_rev_src, ei_rev_dst):
    uslice, gslice, ut, gt = _cfg_derived(cfg)
    ncores = cfg["ncores"]
    cu, cg = cfg["cu"], cfg["cg"]

    x_user = np.ascontiguousarray(np.float32(x_user))
    x_game = np.ascontiguousarray(np.float32(x_game))
    xu_bf = x_user.astype(BF16)
    xg_bf = x_game.astype(BF16)

    def pm_layout(a, n_tiles):
        # [T*P, D] (zero-padded) -> partition-major [P, T*D]
        out = np.zeros((n_tiles * P, a.shape[1]), a.dtype)
        out[: a.shape[0]] = a
        return np.ascontiguousarray(
            out.reshape(n_tiles, P, D).transpose(1, 0, 2).reshape(P, n_tiles * D)
        )

    in_maps = []
    for k in range(ncores):
        idx_u, ld_u, ch_u, r8_u = _pack_side(
            np.asarray(ei_rev_src), np.asarray(ei_rev_dst),
            k * uslice, (k + 1) * uslice, ut, cu,
        )
        idx_g, ld_g, ch_g, r8_g = _pack_side(
            np.asarray(ei_played_src), np.asarray(ei_played_dst),
            k * gslice, (k + 1) * gslice, gt, cg,
        )
        in_maps.append(
            dict(
                xu_bf=xu_bf,
                xg_bf=xg_bf,
                xres_u=pm_layout(x_user[k * uslice : (k + 1) * uslice], ut),
                xres_g=pm_layout(x_game[k * gslice : (k + 1) * gslice], gt),
                idx_u=idx_u, ld_u=ld_u, ch_u=ch_u, r8_u=r8_u,
                idx_g=idx_g, ld_g=ld_g, ch_g=ch_g, r8_g=r8_g,
                w_u=w_user,
                w_g=w_game,
            )
        )
    return in_maps


def _run(inputs, cfg=None, trace=False, **run_kwargs):
    cfg = cfg or CFG_FULL
    uslice, gslice, ut, gt = _cfg_derived(cfg)
    ncores = cfg["ncores"]

    w_user = _fold_weights(
        inputs["Wv_game"], inputs["bv_game"], inputs["Wm_rev"], inputs["bm_rev"],
        inputs["Wout_user"], inputs["bout_user"],
    )
    w_game = _fold_weights(
        inputs["Wv_user"], inputs["bv_user"], inputs["Wm_played"], inputs["bm_played"],
        inputs["Wout_game"], inputs["bout_game"],
    )
    in_maps = _make_in_maps(
        cfg, inputs["x_user"], inputs["x_game"], w_user, w_game,
        inputs["ei_played_src"], inputs["ei_played_dst"],
        inputs["ei_rev_src"], inputs["ei_rev_dst"],
    )
    nc = _get_nc(cfg)
    res = run_bass_kernel_spmd(nc, in_maps, list(range(ncores)), trace=trace, **run_kwargs)

    def unpm(a, n_tiles, nrows):
        # partition-major [P, T*D] -> [T*P, D], trimmed
        return a.reshape(P, n_tiles, D).transpose(1, 0, 2).reshape(n_tiles * P, D)[:nrows]

    out_user = np.concatenate(
        [unpm(res.results[k]["out_u"], ut, uslice) for k in range(ncores)], axis=0
    )
    out_game = np.concatenate(
        [unpm(res.results[k]["out_g"], gt, gslice) for k in range(ncores)], axis=0
    )
    full = np.concatenate([out_user, out_game], axis=0).astype(np.float32)
    return full, res


def kernel(**inputs) -> np.ndarray:
    out, _ = _run(inputs)
    return out



# revision 6
# speedup vs baseline: 1.3979x; 1.3979x over previous
"""HGTConv Trainium2 kernel (8 NeuronCores, dst-sharded edge parallel).

Math: in the reference, softmax over the H=8 head axis followed by
attn.mean(axis=-1) is identically 1/8, so the whole attention branch
(K/Q projections, Wa) drops out:

    out_dst = relu( (segsum_dst(x_src[src]) @ Wbig + cnt*bbig + 8*max(cnt,1)*bout
                     + x_dst*8*max(cnt,1)) / (8*max(cnt,1)) )
    Wbig = Wv @ Wm @ Wout,  bbig = (bv @ Wm + bm) @ Wout

Sharding: each core owns a contiguous dst-node range (1/8 of users +
1/8 of games) and receives exactly the edges pointing into it, so no
collectives are needed. Node features are replicated in bf16.

Device per dst tile (128 nodes): one-hot matmul scatter-add. Edges are
host-packed into chunks of 128; a [128e,128d] selection matrix M
(built on DVE from local-dst values vs an iota row) turns the
segment-sum into PE matmuls accumulating S^T in PSUM. The residual is
added through the PE as identity @ (x*8max(cnt,1)), so the epilogue is
a single fused ACT Relu with per-partition scale 1/(8*max(cnt,1)).

Gathers use the SWDGE dma_gather instruction (int16 indices), one call
per (group of G dst tiles) x (source-table region <= 32768 rows), so
the ~1us SWDGE fixed cost is amortized over ~100-250 KB per call. Each
dst tile owns cap_r chunks per region (caps are data-driven: max needed
across all cores, so the SPMD program is shared); dummy slots gather
region row 0 and are zeroed by the M matrix (ld=-1).
"""

import math
from contextlib import ExitStack

import numpy as np
import ml_dtypes

import concourse.bass as bass
import concourse.tile as tile
import concourse.mybir as mybir
from concourse import bacc
from concourse.bass_utils import run_bass_kernel_spmd

P = 128
D = 256
BF16 = ml_dtypes.bfloat16

# full-size problem config. region boundaries per side: each region width
# must be <= 32768 (int16 gather indices, rebased per region).
CFG_FULL = dict(
    n_user=100000, n_game=50000, ncores=8, gu=7, gg=7,
    bnd_u=(0, 32768, 50000),                    # user side gathers from x_game
    bnd_g=(0, 32768, 65536, 82768, 100000),     # game side gathers from x_user
)


def _cfg_derived(cfg):
    ncores = cfg["ncores"]
    uslice = cfg["n_user"] // ncores
    gslice = cfg["n_game"] // ncores
    ut = math.ceil(uslice / P)
    gt = math.ceil(gslice / P)
    return uslice, gslice, ut, gt


# ----------------------------------------------------------------- host prep

def _region_counts(src, dst, lo, hi, n_tiles, bnd):
    """Per (tile, region) edge counts for one core's dst range."""
    sel = (dst >= lo) & (dst < hi)
    s = src[sel].astype(np.int64)
    d = (dst[sel] - lo).astype(np.int64)
    t = d >> 7
    r = np.searchsorted(np.asarray(bnd), s, side="right") - 1
    R = len(bnd) - 1
    cnt = np.zeros((n_tiles, R), np.int64)
    np.add.at(cnt, (t, r), 1)
    return cnt


def _pack_side_g(src, dst, lo, hi, n_tiles, bnd, caps, G):
    """Pack one core's edges into per-(tile, region) chunk runs.

    Chunk layout (within a group of G tiles): region r occupies chunks
    [G*rb_r, G*(rb_r+cap_r)) of the group; tile ti's run is the cap_r chunks
    starting at G*rb_r + ti*cap_r. Globally, for tile t = g*G+ti:
        pos(t, r, j) = g*G*C + rb_r*G + ti*cap_r + j

    Returns idx16 [P, n_slots//16] int16 (slot s at [s%16, s//16], replicated
    x8 down the partitions; src ids rebased per region, dummies -> 0),
    ld [P, n_chunks] bf16 (ld[p, pos] = local dst of slot (p, pos), dummy ->
    -1), ch [2, n_tiles*P] bf16 (cnt, 8*max(cnt,1)), r8 [P, n_tiles] f32,
    m8 [n_tiles*P] f32.
    """
    R = len(bnd) - 1
    caps = [int(c) for c in caps]
    C = int(sum(caps))
    rb = np.concatenate([[0], np.cumsum(caps)]).astype(np.int64)

    sel = (dst >= lo) & (dst < hi)
    s = src[sel].astype(np.int64)
    d = (dst[sel] - lo).astype(np.int64)
    t = d >> 7
    r = np.searchsorted(np.asarray(bnd), s, side="right") - 1
    order = np.lexsort((d, r, t))
    s, d, t, r = s[order], d[order], t[order], r[order]

    n_chunks = n_tiles * C
    n_slots = n_chunks * P
    idx_flat = np.zeros(n_slots, np.int64)  # dummy -> 0 (region row 0)
    ld = np.full((P, n_chunks), -1.0, np.float32)

    # slot index within a (tile, region) run, via per-(t, r) rank
    key = t * R + r
    # rank within group: counts per key
    uniq, first = np.unique(key, return_index=True)
    rank = np.arange(len(key)) - np.repeat(first, np.diff(np.append(first, len(key))))
    cap_arr = np.asarray(caps)[r] * P
    assert (rank < cap_arr).all(), "per-(tile,region) run overflow"

    g = t // G
    ti = t % G
    pos = g * (G * C) + rb[r] * G + ti * np.asarray(caps)[r] + rank // P
    p = rank % P
    slot = pos * P + p  # slot in "chunk-major, partition-minor" flat space
    idx_flat[slot] = s - np.asarray(bnd)[r]
    ld[p, pos] = (d - (t << 7)).astype(np.float32)

    # 16-wrap per call; calls are per (group, region) covering slots
    # [call_off, call_off + G*caps[r]*128). Within a call, flat entry i
    # (= slot - call_off) lives at partition i%16, column call_off//16 + i//16.
    # Globally this is just: slot -> (partition slot%16, column slot//16),
    # because call offsets are multiples of 128.
    idx16 = np.zeros((16, n_slots // 16), np.int16)
    slots_all = np.arange(n_slots)
    idx16[slots_all % 16, slots_all // 16] = idx_flat
    idx16 = np.tile(idx16, (8, 1))  # replicate across the 8 partition groups

    cnt = np.bincount(d, minlength=n_tiles * P).astype(np.float32)
    m8 = 8.0 * np.maximum(cnt, 1.0)
    ch = np.stack([cnt, m8], axis=0).astype(BF16)
    r8 = np.ascontiguousarray((1.0 / m8).reshape(n_tiles, P).T.astype(np.float32))
    return idx16, ld.astype(BF16), ch, r8, m8


def _fold_weights(Wv, bv, Wm, bm, Wout, bout):
    Wbig = (np.float32(Wv) @ np.float32(Wm)) @ np.float32(Wout)
    bbig = (np.float32(bv) @ np.float32(Wm) + np.float32(bm)) @ np.float32(Wout)
    w = np.concatenate([Wbig, bbig[None, :], np.float32(bout)[None, :]], axis=0)
    return np.ascontiguousarray(w).astype(BF16)  # [D+2, D]


# ------------------------------------------------------------- device build

def _build(cfg, caps_u, caps_g):
    uslice, gslice, ut, gt = _cfg_derived(cfg)
    f32 = mybir.dt.float32
    bf = mybir.dt.bfloat16
    i16 = mybir.dt.int16

    nc = bacc.Bacc(
        "TRN2",
        target_bir_lowering=False,
        debug=False,
        num_devices=cfg["ncores"],
    )

    xu_bf = nc.dram_tensor("xu_bf", [cfg["n_user"], D], bf, kind="ExternalInput")
    xg_bf = nc.dram_tensor("xg_bf", [cfg["n_game"], D], bf, kind="ExternalInput")
    ident_in = nc.dram_tensor("ident", [P, P], bf, kind="ExternalInput")

    sides = []
    for name, tiles, G, xsrc, bnd, caps in (
        ("u", ut, cfg["gu"], xg_bf, cfg["bnd_u"], caps_u),
        ("g", gt, cfg["gg"], xu_bf, cfg["bnd_g"], caps_g),
    ):
        assert tiles % G == 0
        caps = [int(c) for c in caps]
        C = int(sum(caps))
        side = dict(name=name, tiles=tiles, C=C, G=G, xsrc=xsrc, bnd=bnd, caps=caps)
        side["rb"] = [0] + list(np.cumsum(caps))
        side["xm8"] = nc.dram_tensor(f"xm8_{name}", [P, tiles * D], bf, kind="ExternalInput")
        side["idx"] = nc.dram_tensor(
            f"idx_{name}", [P, tiles * C * P // 16], i16, kind="ExternalInput"
        )
        side["ld"] = nc.dram_tensor(f"ld_{name}", [P, tiles * C], bf, kind="ExternalInput")
        side["ch"] = nc.dram_tensor(f"ch_{name}", [2, tiles * P], bf, kind="ExternalInput")
        side["r8"] = nc.dram_tensor(f"r8_{name}", [P, tiles], f32, kind="ExternalInput")
        side["w"] = nc.dram_tensor(f"w_{name}", [D + 2, D], bf, kind="ExternalInput")
        side["out"] = nc.dram_tensor(f"out_{name}", [P, tiles * D], bf, kind="ExternalOutput")
        sides.append(side)

    with tile.TileContext(nc) as tc, ExitStack() as ctx:
        const = ctx.enter_context(tc.tile_pool(name="const", bufs=1))
        gx = ctx.enter_context(tc.tile_pool(name="gx", bufs=3))
        mp = ctx.enter_context(tc.tile_pool(name="mp", bufs=8))
        stp = ctx.enter_context(tc.tile_pool(name="stp", bufs=4))
        xrp = ctx.enter_context(tc.tile_pool(name="xrp", bufs=3))
        outp = ctx.enter_context(tc.tile_pool(name="outp", bufs=3))
        st_ps = ctx.enter_context(tc.tile_pool(name="st_ps", bufs=4, space="PSUM"))
        op_ps = ctx.enter_context(tc.tile_pool(name="op_ps", bufs=3, space="PSUM"))

        # constants
        iota_bf = const.tile([P, P], bf)
        nc.gpsimd.iota(
            iota_bf[:], pattern=[[1, P]], base=0, channel_multiplier=0,
            allow_small_or_imprecise_dtypes=True,
        )
        ident = const.tile([P, P], bf, tag="ident", name="ident_res")
        nc.sync.dma_start(ident[:], ident_in[:])

        for side in sides:
            T, C = side["tiles"], side["C"]
            n = side["name"]
            side["idx_res"] = const.tile(
                [P, T * C * P // 16], i16, tag=f"idx_{n}", name=f"idx_res_{n}"
            )
            nc.sync.dma_start(side["idx_res"][:], side["idx"][:])
            side["ld_res"] = const.tile([P, T * C], bf, tag=f"ld_{n}", name=f"ld_res_{n}")
            nc.sync.dma_start(side["ld_res"][:], side["ld"][:])
            side["ch_res"] = const.tile([2, T * P], bf, tag=f"ch_{n}", name=f"ch_res_{n}")
            nc.sync.dma_start(side["ch_res"][:], side["ch"][:])
            side["r8_res"] = const.tile([P, T], f32, tag=f"r8_{n}", name=f"r8_res_{n}")
            nc.sync.dma_start(side["r8_res"][:], side["r8"][:])
            side["w0"] = const.tile([P, D], bf, tag=f"w0_{n}", name=f"w0_{n}")
            nc.sync.dma_start(side["w0"][:], side["w"][0:P, :])
            side["w1"] = const.tile([P, D], bf, tag=f"w1_{n}", name=f"w1_{n}")
            nc.sync.dma_start(side["w1"][:], side["w"][P : 2 * P, :])
            side["w2"] = const.tile([2, D], bf, tag=f"w2_{n}", name=f"w2_{n}")
            nc.sync.dma_start(side["w2"][:], side["w"][2 * P : 2 * P + 2, :])

        # flat schedule over both sides with a one-tile software-pipeline lag:
        # proj/epilogue of tile t-1 is emitted after the scatter of tile t so
        # the PE never waits on the PSUM->SBUF copies.
        pending = None

        def finish(pend):
            side, t, st0_ps_t, st1_ps_t, xr_g, og_g, gi = pend
            st0 = stp.tile([P, P], bf, tag="st")
            nc.scalar.copy(st0[:], st0_ps_t[:])
            st1 = stp.tile([P, P], bf, tag="st")
            nc.vector.tensor_copy(st1[:], st1_ps_t[:])

            opre = op_ps.tile([P, D], f32, tag="opre")
            nc.tensor.matmul(
                opre[:], lhsT=side["ch_res"][:, t * P : (t + 1) * P],
                rhs=side["w2"][:], start=True, stop=False,
            )
            nc.tensor.matmul(
                opre[:], lhsT=ident[:], rhs=xr_g[:, gi * D : (gi + 1) * D],
                start=False, stop=False,
            )
            nc.tensor.matmul(opre[:], lhsT=st0[:], rhs=side["w0"][:], start=False, stop=False)
            nc.tensor.matmul(opre[:], lhsT=st1[:], rhs=side["w1"][:], start=False, stop=True)

            nc.scalar.activation(
                og_g[:, gi * D : (gi + 1) * D], opre[:],
                mybir.ActivationFunctionType.Relu,
                scale=side["r8_res"][:, t : t + 1],
            )
            if gi == side["G"] - 1:
                g0 = t - side["G"] + 1
                nc.sync.dma_start(side["out"][:, g0 * D : (t + 1) * D], og_g[:])

        for side in sides:
            T, C, G = side["tiles"], side["C"], side["G"]
            caps, rb, bnd = side["caps"], side["rb"], side["bnd"]
            ld_res = side["ld_res"]
            Xg = xr_g = og_g = None
            for t in range(T):
                gi = t % G
                g = t // G
                if gi == 0:
                    # one dma_gather per region for this group of G tiles
                    Xg = gx.tile([P, G * C * D], bf, tag="gx", name="gx_grp")
                    for r in range(len(caps)):
                        if caps[r] == 0:
                            continue
                        ni = G * caps[r] * P
                        slot0 = (g * G * C + rb[r] * G) * P
                        out3 = Xg[:, rb[r] * G * D : (rb[r] + caps[r]) * G * D] \
                            .rearrange("p (c d) -> p c d", d=D)
                        nc.gpsimd.dma_gather(
                            out_ap=out3,
                            in_ap=side["xsrc"][bnd[r] : bnd[r + 1], :],
                            idxs_ap=side["idx_res"][:, slot0 // 16 : (slot0 + ni) // 16],
                            num_idxs=ni,
                            num_idxs_reg=ni,
                            elem_size=D,
                            # one packet caps at 64 descriptors/engine = 1024
                            # idxs; larger calls need multi-packet
                            single_packet=False,
                        )
                    xr_g = xrp.tile([P, G * D], bf, tag="xr", name="xr_grp")
                    nc.sync.dma_start(xr_g[:], side["xm8"][:, t * D : (t + G) * D])
                    og_g = outp.tile([P, G * D], bf, tag="og", name="og_grp")

                # scatter-accumulate S^T for tile t
                st0_ps_t = st_ps.tile([P, P], f32, tag="st")
                st1_ps_t = st_ps.tile([P, P], f32, tag="st")
                ci = 0
                for r in range(len(caps)):
                    for j in range(caps[r]):
                        # chunk position within the group, and global column
                        kpos = rb[r] * G + gi * caps[r] + j
                        col = g * G * C + kpos
                        Mt = mp.tile([P, P], bf, tag="m")
                        nc.vector.tensor_tensor(
                            out=Mt[:],
                            in0=ld_res[:, col : col + 1].to_broadcast([P, P]),
                            in1=iota_bf[:],
                            op=mybir.AluOpType.is_equal,
                        )
                        s_, e_ = (ci == 0), (ci == C - 1)
                        nc.tensor.matmul(
                            st0_ps_t[:], lhsT=Xg[:, kpos * D : kpos * D + P], rhs=Mt[:],
                            start=s_, stop=e_,
                        )
                        nc.tensor.matmul(
                            st1_ps_t[:], lhsT=Xg[:, kpos * D + P : (kpos + 1) * D], rhs=Mt[:],
                            start=s_, stop=e_,
                        )
                        ci += 1

                if pending is not None:
                    finish(pending)
                pending = (side, t, st0_ps_t, st1_ps_t, xr_g, og_g, gi)

        finish(pending)

    nc.compile()
    return nc


_NC_CACHE = {}


def _get_nc(cfg, caps_u, caps_g):
    key = (tuple(sorted((k, tuple(v) if isinstance(v, (tuple, list)) else v)
                        for k, v in cfg.items())), tuple(caps_u), tuple(caps_g))
    if key not in _NC_CACHE:
        _NC_CACHE[key] = _build(cfg, caps_u, caps_g)
    return _NC_CACHE[key]


# ------------------------------------------------------------------- driver

def _compute_caps(cfg, ei_played_src, ei_played_dst, ei_rev_src, ei_rev_dst):
    """Data-driven per-(region) chunk caps: max over all cores and tiles."""
    uslice, gslice, ut, gt = _cfg_derived(cfg)
    ncores = cfg["ncores"]
    cnts_u = np.max(
        [_region_counts(np.asarray(ei_rev_src), np.asarray(ei_rev_dst),
                        k * uslice, (k + 1) * uslice, ut, cfg["bnd_u"])
         for k in range(ncores)], axis=(0, 1),
    )
    cnts_g = np.max(
        [_region_counts(np.asarray(ei_played_src), np.asarray(ei_played_dst),
                        k * gslice, (k + 1) * gslice, gt, cfg["bnd_g"])
         for k in range(ncores)], axis=(0, 1),
    )
    caps_u = [int(math.ceil(c / P)) for c in cnts_u]
    caps_g = [int(math.ceil(c / P)) for c in cnts_g]
    return caps_u, caps_g


def _make_in_maps(cfg, caps_u, caps_g, x_user, x_game, w_user, w_game,
                  ei_played_src, ei_played_dst, ei_rev_src, ei_rev_dst):
    uslice, gslice, ut, gt = _cfg_derived(cfg)
    ncores = cfg["ncores"]

    x_user = np.ascontiguousarray(np.float32(x_user))
    x_game = np.ascontiguousarray(np.float32(x_game))
    xu_bf = x_user.astype(BF16)
    xg_bf = x_game.astype(BF16)
    ident = np.eye(P, dtype=BF16)

    def pm_layout_m8(x_slice, m8, n_tiles):
        out = np.zeros((n_tiles * P, D), np.float32)
        out[: x_slice.shape[0]] = x_slice * m8[: x_slice.shape[0], None]
        return np.ascontiguousarray(
            out.reshape(n_tiles, P, D).transpose(1, 0, 2).reshape(P, n_tiles * D)
        ).astype(BF16)

    in_maps = []
    for k in range(ncores):
        idx_u, ld_u, ch_u, r8_u, m8_u = _pack_side_g(
            np.asarray(ei_rev_src), np.asarray(ei_rev_dst),
            k * uslice, (k + 1) * uslice, ut, cfg["bnd_u"], caps_u, cfg["gu"],
        )
        idx_g, ld_g, ch_g, r8_g, m8_g = _pack_side_g(
            np.asarray(ei_played_src), np.asarray(ei_played_dst),
            k * gslice, (k + 1) * gslice, gt, cfg["bnd_g"], caps_g, cfg["gg"],
        )
        in_maps.append(
            dict(
                xu_bf=xu_bf,
                xg_bf=xg_bf,
                ident=ident,
                xm8_u=pm_layout_m8(x_user[k * uslice : (k + 1) * uslice], m8_u, ut),
                xm8_g=pm_layout_m8(x_game[k * gslice : (k + 1) * gslice], m8_g, gt),
                idx_u=idx_u, ld_u=ld_u, ch_u=ch_u, r8_u=r8_u,
                idx_g=idx_g, ld_g=ld_g, ch_g=ch_g, r8_g=r8_g,
                w_u=w_user,
                w_g=w_game,
            )
        )
    return in_maps


def _run(inputs, cfg=None, trace=False, **run_kwargs):
    cfg = cfg or CFG_FULL
    uslice, gslice, ut, gt = _cfg_derived(cfg)
    ncores = cfg["ncores"]

    w_user = _fold_weights(
        inputs["Wv_game"], inputs["bv_game"], inputs["Wm_rev"], inputs["bm_rev"],
        inputs["Wout_user"], inputs["bout_user"],
    )
    w_game = _fold_weights(
        inputs["Wv_user"], inputs["bv_user"], inputs["Wm_played"], inputs["bm_played"],
        inputs["Wout_game"], inputs["bout_game"],
    )
    caps_u, caps_g = _compute_caps(
        cfg, inputs["ei_played_src"], inputs["ei_played_dst"],
        inputs["ei_rev_src"], inputs["ei_rev_dst"],
    )
    in_maps = _make_in_maps(
        cfg, caps_u, caps_g, inputs["x_user"], inputs["x_game"], w_user, w_game,
        inputs["ei_played_src"], inputs["ei_played_dst"],
        inputs["ei_rev_src"], inputs["ei_rev_dst"],
    )
    nc = _get_nc(cfg, caps_u, caps_g)
    res = run_bass_kernel_spmd(nc, in_maps, list(range(ncores)), trace=trace, **run_kwargs)

    def unpm(a, n_tiles, nrows):
        a = np.float32(a)
        return a.reshape(P, n_tiles, D).transpose(1, 0, 2).reshape(n_tiles * P, D)[:nrows]

    out_user = np.concatenate(
        [unpm(res.results[k]["out_u"], ut, uslice) for k in range(ncores)], axis=0
    )
    out_game = np.concatenate(
        [unpm(res.results[k]["out_g"], gt, gslice) for k in range(ncores)], axis=0
    )
    full = np.concatenate([out_user, out_game], axis=0).astype(np.float32)
    return full, res


def kernel(**inputs) -> np.ndarray:
    out, _ = _run(inputs)
    return out
